# revision 1
# baseline (speedup 1.0000x reference)
"""Trainium2 Bass kernel for DiagTrainableLDAHead (retrieval_knn).

out[n,c] = log_prior[c] - 0.5*(m2[n,c] + log_det)
m2[n,c]  = sum_d (z[n,d]-mu[c,d])^2 * inv_var[d]
         = z_sq[n] - 2*cross[n,c] + mu_sq[c]

=> out[n,c] = cross[n,c] + rb[n] + cb[c]
   cross = z @ (mu * inv_var).T            (GEMM, fp32r single-pass)
   rb[n] = -0.5 * sum_d z[n,d]^2 inv_var[d]
   cb[c] = log_prior[c] - 0.5*(mu_sq[c] + log_det)

Sharding: data-parallel over N across 8 NeuronCores (1024 rows each);
mu / log_cov_diag / prior_logits replicated. Forward-only: no collectives.
Host prep is layout-only (transposes so the contraction dim D sits on
SBUF partitions for both GEMM operands); all arithmetic is on-device.

Inputs stream in as column chunks (full D for a c- or n-range), chained
so early chunks complete early and the GEMM overlaps the load.
"""
import sys

sys.path.insert(0, "/opt/trn_rl_repo")

import numpy as np

import concourse.bacc as bacc
import concourse.tile as tile
from concourse import mybir
from concourse.bass_utils import run_bass_kernel_spmd

F32 = mybir.dt.float32
F32R = mybir.dt.float32r
AF = mybir.ActivationFunctionType
ALU = mybir.AluOpType

N, C, D = 8192, 2048, 512
NCORES = 8
NSH = N // NCORES          # 1024 rows per core
P = 128                    # partitions
KT = D // P                # 4 k-tiles
NT = NSH // P              # 8 n-tiles
F = 512                    # c-chunk (PSUM bank / fp32 moving max)
CJ = C // F                # 4 c-chunks

_CACHE = {}


def _build():
    nc = bacc.Bacc("TRN2", target_bir_lowering=False, debug=False,
                   enable_asserts=False, num_devices=NCORES)

    # z/mu arrive host-pre-rounded to fp32r (FP22) — still 4-byte fp32
    # bits — so DMA can feed the fp32r GEMM operands directly.
    zT = nc.dram_tensor("zT", [D, NSH], F32R, kind="ExternalInput").ap()
    muT = nc.dram_tensor("muT", [D, C], F32R, kind="ExternalInput").ap()
    lc = nc.dram_tensor("lc", [D], F32, kind="ExternalInput").ap()
    prior = nc.dram_tensor("prior", [C], F32, kind="ExternalInput").ap()
    out = nc.dram_tensor("out", [NSH, C], F32, kind="ExternalOutput").ap()

    with tile.TileContext(nc) as tc:
        with (
            tc.tile_pool(name="const", bufs=1) as const,
            tc.tile_pool(name="sq", bufs=2) as sq,
            tc.tile_pool(name="stage", bufs=3) as stage,
            tc.tile_pool(name="psS", bufs=2, space="PSUM") as psS,
            tc.tile_pool(name="psZ", bufs=2, space="PSUM") as psZ,
            tc.tile_pool(name="psM", bufs=4, space="PSUM") as psM,
        ):
            # ---- small constants --------------------------------------
            # (issued on the scalar queue so the sync queue's first issue
            # is the first big mu chunk)
            lc_f = const.tile([1, D], F32)
            nc.scalar.dma_start(out=lc_f[:], in_=lc.rearrange("(a d) -> a d", a=1))
            pr = const.tile([1, C], F32)
            nc.scalar.dma_start(out=pr[:], in_=prior.rearrange("(a c) -> a c", a=1))

            # log_cov in partition layout [p, t] (d = t*128 + p) via PE
            # transposes — a strided DMA gather here costs ~3us of
            # descriptor generation on the sequencer.
            id1 = const.tile([1, 1], F32)
            nc.vector.memset(id1[:], 1.0)
            plc = psZ.tile([P, KT], F32, tag="zchain")
            for kt in range(KT):
                nc.tensor.transpose(plc[:, kt:kt + 1],
                                    lc_f[:, kt * P:(kt + 1) * P], id1[:])
            lc_p = const.tile([P, KT], F32)
            nc.scalar.copy(lc_p[:], plc[:])

            iv = const.tile([P, KT], F32)      # exp(-lc), for scalar ops
            nc.scalar.activation(iv[:], lc_p[:], AF.Exp, scale=-1.0)
            iv_r = const.tile([P, KT], F32R)   # rounded copy, matmul operand
            nc.scalar.activation(iv_r[:], lc_p[:], AF.Exp, scale=-1.0)

            # log_det = sum(lc); computed exactly along the free dim
            ldsum = const.tile([1, 1], F32)
            nc.vector.tensor_reduce(out=ldsum[:], in_=lc_f[:],
                                    axis=mybir.AxisListType.X, op=ALU.add)
            nldh = const.tile([1, 1], F32)     # -0.5 * log_det
            nc.scalar.mul(nldh[:], ldsum[:], -0.5)

            # log_prior = prior - max - log(sum(exp(prior - max)))
            pmax = const.tile([1, 1], F32)
            nc.vector.tensor_reduce(out=pmax[:], in_=pr[:],
                                    axis=mybir.AxisListType.X, op=ALU.max)
            npmax = const.tile([1, 1], F32)
            nc.scalar.mul(npmax[:], pmax[:], -1.0)
            pexp = const.tile([1, C], F32)
            nc.scalar.activation(pexp[:], pr[:], AF.Exp, bias=npmax[:], scale=1.0)
            psum_e = const.tile([1, 1], F32)
            nc.vector.tensor_reduce(out=psum_e[:], in_=pexp[:],
                                    axis=mybir.AxisListType.X, op=ALU.add)
            lse = const.tile([1, 1], F32)
            nc.scalar.activation(lse[:], psum_e[:], AF.Ln)
            nb = const.tile([1, 1], F32)       # -(lse + pmax)
            nc.scalar.activation(nb[:], lse[:], AF.Identity, bias=pmax[:], scale=1.0)
            nc.scalar.mul(nb[:], nb[:], -1.0)
            lp = const.tile([1, C], F32)       # log_prior
            nc.scalar.activation(lp[:], pr[:], AF.Identity, bias=nb[:], scale=1.0)

            ones_f = const.tile([1, P], F32)
            nc.vector.memset(ones_f[:], 1.0)
            ones1 = const.tile([1, P], F32R)
            nc.scalar.copy(ones1[:], ones_f[:])

            # ---- streamed loads + per-chunk preprocess ----------------
            # Every input chunk DMAs into its own persistent slice so no
            # DMA ever waits on a pool slot (a waiting dma_start
            # head-of-line-blocks the sync sequencer's later issues).
            muT_s = const.tile([P, KT, C], F32R)    # mu^T (pre-rounded)
            zF = const.tile([P, KT, NSH], F32R)     # z^T (pre-rounded)
            zT_s = const.tile([P, KT, NSH], F32R)   # z^T * inv_var
            eRt = const.tile([1, C], F32)
            eR = const.tile([1, C], F32R)
            cb = const.tile([P, C], F32)            # eR broadcast to partitions
            rb = const.tile([P, NT], F32)           # -0.5 * z_sq
            zsqf = const.tile([1, NSH], F32)        # -0.5 * z_sq, free layout
            ZW = 2 * P                              # z column-chunk width

            def load_mu(cj):
                s = slice(cj * F, (cj + 1) * F)
                nc.sync.dma_start(out=muT_s[:, :, s],
                                  in_=muT[:, s]
                                  .rearrange("(t p) c -> p t c", p=P))
                sqm = sq.tile([P, KT, F], F32R, tag="sqm")
                nc.scalar.activation(sqm[:], muT_s[:, :, s], AF.Square)
                with tc.high_priority():
                    pmu = psS.tile([P, F], F32, tag="setup")
                    for kt in range(KT):
                        nc.tensor.matmul(pmu[0:1, :], lhsT=iv_r[:, kt:kt + 1],
                                         rhs=sqm[:, kt, :],
                                         start=(kt == 0), stop=(kt == KT - 1))
                    # eR[c] = log_prior[c] - 0.5*(mu_sq[c] + log_det)
                    nc.scalar.activation(eRt[:, s], pmu[0:1, :],
                                         AF.Identity, bias=nldh[:], scale=-0.5)
                    nc.vector.tensor_tensor(eR[:, s], eRt[:, s], lp[:, s],
                                            ALU.add)
                    # broadcast to all partitions via rank-1 matmul
                    pc = psS.tile([P, F], F32, tag="setup")
                    nc.tensor.matmul(pc[:], lhsT=ones1[:], rhs=eR[:, s],
                                     start=True, stop=True)
                    nc.scalar.copy(cb[:, s], pc[:])

            def load_z(zi):
                s = slice(zi * ZW, (zi + 1) * ZW)
                nc.sync.dma_start(out=zF[:, :, s],
                                  in_=zT[:, s]
                                  .rearrange("(t p) c -> p t c", p=P))
                for kt in range(KT):
                    nc.vector.tensor_scalar_mul(
                        zT_s[:, kt, s], zF[:, kt, s], iv[:, kt:kt + 1])
                zq = sq.tile([P, KT, ZW], F32R, tag="zq")
                nc.vector.tensor_tensor(zq[:], zF[:, :, s], zF[:, :, s],
                                        ALU.mult)
                # z_sq in free layout: [1, ZW] psum chunk, then PE-transpose
                # each 128-wide piece into partition layout for the evict bias
                with tc.high_priority():
                    pzf = psZ.tile([P, ZW], F32, tag="zchain")
                    for kt in range(KT):
                        nc.tensor.matmul(pzf[0:1, :], lhsT=iv_r[:, kt:kt + 1],
                                         rhs=zq[:, kt, :],
                                         start=(kt == 0), stop=(kt == KT - 1))
                    nc.scalar.activation(zsqf[:, s], pzf[0:1, :], AF.Copy,
                                         scale=-0.5)
                    prb = psZ.tile([P, 2], F32, tag="zchain")
                    for lni in range(ZW // P):
                        ni = zi * (ZW // P) + lni
                        nc.tensor.transpose(prb[:, lni:lni + 1],
                                            zsqf[:, ni * P:(ni + 1) * P], id1[:])
                    nc.scalar.copy(rb[:, zi * 2:zi * 2 + 2], prb[:])

            # ---- main GEMM: out = cross + rb + cb ---------------------
            def main_tile(ni):
                ot = stage.tile([P, C], F32)
                for cj in range(CJ):
                    ps = psM.tile([P, F], F32)
                    for kt in range(KT):
                        nc.tensor.matmul(
                            ps[:],
                            lhsT=zT_s[:, kt, ni * P:(ni + 1) * P],
                            rhs=muT_s[:, kt, cj * F:(cj + 1) * F],
                            start=(kt == 0), stop=(kt == KT - 1))
                    nc.scalar.activation(ot[:, cj * F:(cj + 1) * F], ps[:],
                                         AF.Identity, bias=rb[:, ni:ni + 1],
                                         scale=1.0)
                    nc.vector.tensor_tensor(ot[:, cj * F:(cj + 1) * F],
                                            ot[:, cj * F:(cj + 1) * F],
                                            cb[:, cj * F:(cj + 1) * F], ALU.add)
                    # drain the last tiles' output per chunk so the final
                    # transfer after the last evict is as small as possible
                    if ni >= NT - 2:
                        nc.sync.dma_start(
                            out=out[ni * P:(ni + 1) * P, cj * F:(cj + 1) * F],
                            in_=ot[:, cj * F:(cj + 1) * F])
                if ni < NT - 2:
                    nc.sync.dma_start(out=out[ni * P:(ni + 1) * P, :], in_=ot[:])

            for i in range(CJ):
                load_mu(i)
                load_z(i)
            for ni in range(NT):
                main_tile(ni)

    nc.compile()
    return nc


def _get_nc():
    if "nc" not in _CACHE:
        _CACHE["nc"] = _build()
    return _CACHE["nc"]


def _round_f32r(a):
    """Round fp32 to fp32r/FP22 (13-bit mantissa, RNE) — dtype prep for the
    fp32r-typed DRAM operands, same spirit as casting an input to bf16."""
    u = np.ascontiguousarray(a, dtype=np.float32).view(np.uint32)
    r = (u + np.uint32(0x1FF) + ((u >> np.uint32(10)) & np.uint32(1))) \
        & np.uint32(0xFFFFFC00)
    return r.view(np.float32)


def _in_maps(z, mu, log_cov_diag, prior_logits):
    z = np.ascontiguousarray(np.asarray(z, dtype=np.float32))
    mu = np.asarray(mu, dtype=np.float32)
    lc = np.ascontiguousarray(np.asarray(log_cov_diag, dtype=np.float32))
    pl = np.ascontiguousarray(np.asarray(prior_logits, dtype=np.float32))
    muT = _round_f32r(np.ascontiguousarray(mu.T))
    maps = []
    for c in range(NCORES):
        zTc = _round_f32r(np.ascontiguousarray(z[c * NSH:(c + 1) * NSH, :].T))
        maps.append({"zT": zTc, "muT": muT, "lc": lc, "prior": pl})
    return maps


def _run(z, mu, log_cov_diag, prior_logits, trace=False, **kw):
    nc = _get_nc()
    maps = _in_maps(z, mu, log_cov_diag, prior_logits)
    res = run_bass_kernel_spmd(nc, maps, list(range(NCORES)), trace=trace, **kw)
    full = np.concatenate([res.results[c]["out"] for c in range(NCORES)], axis=0)
    return full, res


def kernel(z, mu, log_cov_diag, prior_logits):
    full, _ = _run(z, mu, log_cov_diag, prior_logits)
    return full

